# revision 35
# baseline (speedup 1.0000x reference)
"""Trainium2 Bass kernel for nn_AttentionModule (channel self-attention).

Reference computation (per batch sample b, with x: [C=512, N=4096]):
    q   = w1 @ x + b1                     # [64, 4096]
    att = softmax(q @ q.T, axis=-1)       # [64, 64]
    out = att @ q                         # [64, 4096]
    y   = w2 @ out + b2 + x               # [512, 4096]

Sharding: data-parallel over batch. B=16 samples, 8 cores, 2 samples/core.
Small weights (w1,b1,w2,b2) replicated to every core (host pre-transposes
them into matmul-friendly layouts: w1t=[C,Cr], w2t=[Cr,C], row vectors).

Key structure (vs the earlier 145us version, which was PE-bound and ran
the tensor engine at the HAM-throttled 1.2 GHz clock for most of the
kernel while the final sample's output phase trickled stores):

  - w2 @ (att @ q) == (w2 @ att) @ q: compute m2T = att^T @ w2^T with a
    single N=512 matmul (att stationary), then y = m2aug @ q_aug where
    m2aug = [m2^T; b2] (K=65).  This removes the whole out=att@q pass
    and its PSUM evacuations.
  - b1 enters q via a K=1 matmul (b1 row stationary, ones row moving),
    so the q evacuation is a pure ACT cast to bf16.
  - q, qT, att, m2, w2 all bf16 (softmax is saturated: att diagonal
    dominates by >2000, and the 2e-2 tolerance leaves 100x margin).
  - PE transposes of q go to a bf16 PSUM tile, 8 slices per bank, one
    [128,8,64] ACT evacuation each (instead of 32 per-slice copies).
  - software pipelining: per x piece p the PE runs [q-MMs p],
    [transposes p-1], [gram p-2] so no PE instruction waits on an
    evacuation that was just issued; sample 1's stream phase is
    interleaved with sample 0's output phase in 2-matmul blocks so the
    PE queue never head-blocks on the DVE-paced residual adds.
  - engine budget per core: PE ~88K cycles, DVE ~45us (residual adds +
    softmax), ACT ~20us (evacuations), DMA 94us (16.8MB in + 16.8MB
    out at ~358 GB/s) -- DMA is the roofline; everything else hides
    under it.
"""

import os
import sys
from contextlib import ExitStack

import ml_dtypes
import numpy as np

for _p in ("/opt/trn_rl_repo", "/root/.axon_site/_ro/trn_rl_repo"):
    if os.path.isdir(_p) and _p not in sys.path:
        sys.path.append(_p)

import concourse.bass as bass  # noqa: E402
import concourse.tile as tile  # noqa: E402
from concourse import bacc, mybir  # noqa: E402
from concourse.bass_utils import run_bass_kernel_spmd  # noqa: E402

F32 = mybir.dt.float32
F32R = mybir.dt.float32r
BF16 = mybir.dt.bfloat16
AF = mybir.ActivationFunctionType
ALU = mybir.AluOpType
AX = mybir.AxisListType

B, C, CR = 16, 512, 64
W, H = 64, 64
N = W * H  # 4096
NCORES = 8
BPC = B // NCORES  # samples per core
KC = C // 128  # 4 k-chunks of x / oc-chunks of output
NF = 512  # n-chunk (PSUM bank) width
NN = N // NF  # 8 n-chunks per sample
LF = 1024  # DMA piece width (load and store)
NL = N // LF  # 4 pieces per sample row
NPP = LF // NF  # 2 n-chunks per piece
TSL = LF // 128  # 8 transpose slices per piece
NT = N // 128  # 32 transpose slices per sample


def _build_nc():
    nc = bacc.Bacc(
        "TRN2",
        target_bir_lowering=False,
        debug=False,
        enable_asserts=True,
        num_devices=NCORES,
    )
    x_d = nc.dram_tensor("x", [BPC, C, N], F32, kind="ExternalInput").ap()
    w1t_d = nc.dram_tensor("w1t", [C, CR], F32, kind="ExternalInput").ap()
    b1c_d = nc.dram_tensor("b1c", [CR, 1], F32, kind="ExternalInput").ap()
    w2t_d = nc.dram_tensor("w2t", [CR, C], F32, kind="ExternalInput").ap()
    b2r_d = nc.dram_tensor("b2r", [1, C], F32, kind="ExternalInput").ap()
    identb_d = nc.dram_tensor("identb", [CR, CR], BF16, kind="ExternalInput").ap()
    onesb_d = nc.dram_tensor("onesb", [1, N], BF16, kind="ExternalInput").ap()
    out_d = nc.dram_tensor("out", [BPC, C, N], F32, kind="ExternalOutput").ap()

    with tile.TileContext(nc) as tc, ExitStack() as ctx:
        singles = ctx.enter_context(tc.tile_pool(name="singles", bufs=1))
        fin = ctx.enter_context(tc.tile_pool(name="fin", bufs=6))
        small = ctx.enter_context(tc.tile_pool(name="small", bufs=2))
        ps_q = ctx.enter_context(tc.tile_pool(name="ps_q", bufs=2, space="PSUM"))
        ps_tp = ctx.enter_context(tc.tile_pool(name="ps_tp", bufs=2, space="PSUM"))
        ps_att = ctx.enter_context(tc.tile_pool(name="ps_att", bufs=1, space="PSUM"))
        ps_o = ctx.enter_context(tc.tile_pool(name="ps_o", bufs=3, space="PSUM"))

        # ---------- constants / weight prep ----------
        # w1t chunks first (first q matmul needs them), then x piece 0
        # is issued before the rest of the prologue weights.
        w1t_st = singles.tile([128, KC, CR], F32, tag="w1t_st")
        nc.sync.dma_start(out=w1t_st, in_=w1t_d.rearrange("(k p) m -> p k m", k=KC))
        # x tiles: one [128, KC, 1024] 2MB dma per piece; piece (0,0) is
        # issued before the rest of the prologue so compute starts early.
        xts = [
            singles.tile([128, KC, N], BF16, tag=f"x{s}", name=f"xt{s}")
            for s in range(BPC)
        ]

        def load_piece(s, piece):
            """SWDGE load with inline f32 -> bf16 cast (gpsimd-only feature)."""
            lsl = bass.ts(piece, LF)
            nc.gpsimd.dma_start(
                out=xts[s][:, :, lsl],
                in_=x_d[s, :, lsl].rearrange("(k p) n -> p k n", p=128),
            )

        # all x pieces issue back-to-back at the head of the gpsimd queue
        for piece in range(NL):
            load_piece(0, piece)
        for piece in range(NL):
            load_piece(1, piece)

        ident = singles.tile([CR, CR], BF16, tag="ident")
        nc.sync.dma_start(out=ident, in_=identb_d)
        w1t_bf = singles.tile([128, KC, CR], BF16, tag="w1t_bf")
        nc.vector.tensor_copy(w1t_bf, w1t_st)
        b1c_sb = singles.tile([CR, 1], F32, tag="b1c")
        nc.sync.dma_start(out=b1c_sb, in_=b1c_d)
        w2t_st = singles.tile([CR, C], F32, tag="w2t_st")
        nc.sync.dma_start(out=w2t_st, in_=w2t_d)
        w2t_bf = singles.tile([CR, C], BF16, tag="w2t_bf")
        nc.vector.tensor_copy(w2t_bf, w2t_st)
        b2_st = small.tile([1, C], F32, tag="b2st")
        nc.sync.dma_start(out=b2_st, in_=b2r_d)
        # per-sample persistent tiles
        m2aug = []
        q_aug = []
        qT = []
        for s in range(BPC):
            ma = singles.tile([CR + 1, C], BF16, tag=f"m2aug{s}")
            nc.vector.tensor_copy(ma[CR : CR + 1, :], b2_st)
            m2aug.append(ma)
            qa = singles.tile([CR + 1, N], BF16, tag=f"qaug{s}")
            nc.sync.dma_start(out=qa[CR : CR + 1, :], in_=onesb_d)
            q_aug.append(qa)
            qt = singles.tile([128, NT, CR], BF16, tag=f"qT{s}", name=f"qT{s}")
            qT.append(qt)
        # warm the Exp table set early (off the critical path)
        _warm = small.tile([1, 1], F32, tag="warm")
        nc.scalar.activation(_warm, b2_st[:, 0:1], AF.Exp, bias=0.0, scale=0.0)

        state = {}

        def begin_sample(s):
            state[s] = {
                "patt": ps_att.tile([CR, CR], F32, tag="patt", name=f"patt{s}"),
            }

        def mm_chunk(s, n):
            """q[:, n] = w1 @ x into PSUM; ACT-evacuate (+b1 bias) as bf16."""
            nsl = bass.ts(n, NF)
            pq = ps_q.tile([CR, NF], F32, tag="pq", name=f"pq{s}_{n}")
            for k in range(KC):
                nc.tensor.matmul(
                    pq,
                    w1t_bf[:, k, :],
                    xts[s][:, k, nsl],
                    start=(k == 0),
                    stop=(k == KC - 1),
                )
            nc.scalar.activation(
                q_aug[s][0:CR, nsl], pq, AF.Identity, bias=b1c_sb, scale=1.0
            )

        def tp_piece(s, p):
            """PE-transpose 8 q slices of piece p into one bf16 PSUM bank."""
            tpt = ps_tp.tile([128, TSL, CR], BF16, tag="tp", name=f"tp{s}_{p}")
            for sl in range(TSL):
                t = p * TSL + sl
                nc.tensor.transpose(
                    tpt[:, sl, :],
                    q_aug[s][0:CR, t * 128 : (t + 1) * 128],
                    ident,
                )
            nc.scalar.copy(qT[s][:, p * TSL : (p + 1) * TSL, :], tpt)

        def gram_piece(s, p):
            for sl in range(TSL):
                t = p * TSL + sl
                qs = qT[s][:, t, :]
                nc.tensor.matmul(
                    state[s]["patt"], qs, qs, start=(t == 0), stop=(t == NT - 1)
                )

        def softmax_m2(s):
            """patt -> att (softmax) -> m2aug rows 0..63 = (w2 @ att)^T."""
            patt = state[s]["patt"]
            negm = small.tile([CR, 1], F32, tag="negm", name=f"negm{s}")
            nc.vector.tensor_reduce(
                out=negm, in_=patt, axis=AX.X, op=ALU.max, negate=True
            )
            shifted = small.tile([CR, CR], F32, tag="shifted", name=f"shifted{s}")
            nc.vector.tensor_scalar(
                out=shifted, in0=patt, scalar1=negm, scalar2=-80.0,
                op0=ALU.add, op1=ALU.max,
            )
            atte = small.tile([CR, CR], F32, tag="atte", name=f"atte{s}")
            ssum = small.tile([CR, 1], F32, tag="ssum", name=f"ssum{s}")
            nc.scalar.activation(
                atte, shifted, AF.Exp, bias=0.0, scale=1.0, accum_out=ssum
            )
            rsum = small.tile([CR, 1], F32, tag="rsum", name=f"rsum{s}")
            nc.vector.reciprocal(rsum, ssum)
            attn = small.tile([CR, CR], BF16, tag="attn", name=f"attn{s}")
            nc.vector.tensor_scalar_mul(attn, atte, rsum)
            # m2T = att^T @ w2^T : one matmul, att stationary
            pm2 = ps_q.tile([CR, C], F32, tag="pq", name=f"pm2{s}")
            nc.tensor.matmul(pm2, attn, w2t_bf, start=True, stop=True)
            nc.scalar.copy(m2aug[s][0:CR, :], pm2)

        def y_block(s, j):
            """y chunk: 2 matmuls + residual adds + one [128,1024] store.

            Runs at scheduler priority 0 so the store-feeding chain is
            popped the moment its dependencies resolve, instead of after
            the whole remaining stream phase."""
            oc, half = divmod(j, NL)
            osl = slice(oc * 128, (oc + 1) * 128)
            with tc.high_priority():
                f = fin.tile([128, LF], F32, tag="fin", name=f"fin{s}_{j}")
                for sub in range(NPP):
                    n = half * NPP + sub
                    nsl = bass.ts(n, NF)
                    py = ps_o.tile([128, NF], F32, tag="py", name=f"py{s}_{oc}_{n}")
                    nc.tensor.matmul(
                        py, m2aug[s][:, osl], q_aug[s][:, nsl], start=True, stop=True
                    )
                    nc.vector.tensor_add(
                        f[:, sub * NF : (sub + 1) * NF], py, xts[s][:, oc, nsl]
                    )
                nc.sync.dma_start(out=out_d[s, osl, bass.ts(half, LF)], in_=f)

        def a_blocks(s):
            """Stream-phase PE blocks for sample s, software-pipelined."""
            blocks = []
            done_tp = [False] * NL
            done_gr = [False] * NL
            for p in range(NL):
                blocks.append(lambda s=s, n=2 * p: mm_chunk(s, n))
                blocks.append(lambda s=s, n=2 * p + 1: mm_chunk(s, n))
                if p >= 1:
                    blocks.append(lambda s=s, p=p - 1: tp_piece(s, p))
                    done_tp[p - 1] = True
                if p >= 2:
                    blocks.append(lambda s=s, p=p - 2: gram_piece(s, p))
                    done_gr[p - 2] = True
            for p in range(NL):
                if not done_tp[p]:
                    blocks.append(lambda s=s, p=p: tp_piece(s, p))
            for p in range(NL):
                if not done_gr[p]:
                    blocks.append(lambda s=s, p=p: gram_piece(s, p))
            return blocks

        def warm_burst(tag, nmm=16):
            """Back-to-back dependency-free bf16 matmuls to trip the HAM
            activity monitor into K=8/8 (2.4 GHz).  Output is discarded.
            Uses the pq pool (idle between stream phases) and normal
            priority, so real work always preempts it."""
            for i in range(nmm):
                pw = ps_q.tile([CR, NF], F32, tag="pq", name=f"warm{tag}_{i}")
                nc.tensor.matmul(
                    pw, w2t_bf[:, 0:CR], w2t_bf, start=True, stop=True
                )

        # ---------- schedule ----------
        begin_sample(0)
        for blk in a_blocks(0):
            blk()
        softmax_m2(0)
        begin_sample(1)
        warm_burst("i")
        s1_blocks = a_blocks(1)
        s0_yblocks = [lambda s=0, j=j: y_block(s, j) for j in range(16)]
        # interleave: alternate one s1 stream block with one s0 y block
        for i in range(max(len(s1_blocks), len(s0_yblocks))):
            if i < len(s1_blocks):
                s1_blocks[i]()
            if i < len(s0_yblocks):
                s0_yblocks[i]()
        softmax_m2(1)
        warm_burst("t", nmm=24)
        for j in range(16):
            y_block(1, j)

    nc.compile()
    return nc


_NC_CACHE = None


def _get_nc():
    global _NC_CACHE
    if _NC_CACHE is None:
        _NC_CACHE = _build_nc()
    return _NC_CACHE


def _as_f32(a):
    return np.ascontiguousarray(np.asarray(a, dtype=np.float32))


def run(inputs, trace=False):
    """Run on all 8 cores; returns (full output [B,C,W,H], BassKernelResults)."""
    nc = _get_nc()
    x = _as_f32(inputs["x"]).reshape(B, C, N)
    w1t = _as_f32(np.asarray(inputs["w1"]).T)  # [C, CR]
    b1c = _as_f32(inputs["b1"]).reshape(CR, 1)
    w2t = _as_f32(np.asarray(inputs["w2"]).T)  # [CR, C]
    b2r = _as_f32(inputs["b2"]).reshape(1, C)
    in_maps = [
        {
            "x": x[c * BPC : (c + 1) * BPC],
            "w1t": w1t,
            "b1c": b1c,
            "w2t": w2t,
            "b2r": b2r,
            "identb": np.eye(CR, dtype=ml_dtypes.bfloat16),
            "onesb": np.ones((1, N), dtype=ml_dtypes.bfloat16),
        }
        for c in range(NCORES)
    ]
    res = run_bass_kernel_spmd(nc, in_maps, list(range(NCORES)), trace=trace)
    out = np.concatenate([res.results[c]["out"] for c in range(NCORES)], axis=0)
    return out.reshape(B, C, W, H).astype(np.float32, copy=False), res


def kernel(**inputs):
    out, _ = run(inputs)
    return out


# revision 38
# speedup vs baseline: 1.1347x; 1.1347x over previous
"""Trainium2 Bass kernel for nn_AttentionModule (channel self-attention).

Reference computation (per batch sample b, with x: [C=512, N=4096]):
    q   = w1 @ x + b1                     # [64, 4096]
    att = softmax(q @ q.T, axis=-1)       # [64, 64]
    out = att @ q                         # [64, 4096]
    y   = w2 @ out + b2 + x               # [512, 4096]

Sharding: data-parallel over batch. B=16 samples, 8 cores, 2 samples/core.
Small weights (w1,b1,w2,b2) replicated to every core (host pre-transposes
them into matmul-friendly layouts: w1t=[C,Cr], w2t=[Cr,C], row vectors).

Key structure (vs the earlier 145us version, which was PE-bound and ran
the tensor engine at the HAM-throttled 1.2 GHz clock for most of the
kernel while the final sample's output phase trickled stores):

  - w2 @ (att @ q) == (w2 @ att) @ q: compute m2T = att^T @ w2^T with a
    single N=512 matmul (att stationary), then y = m2aug @ q_aug where
    m2aug = [m2^T; b2] (K=65).  This removes the whole out=att@q pass
    and its PSUM evacuations.
  - b1 enters q via a K=1 matmul (b1 row stationary, ones row moving),
    so the q evacuation is a pure ACT cast to bf16.
  - q, qT, att, m2, w2 all bf16 (softmax is saturated: att diagonal
    dominates by >2000, and the 2e-2 tolerance leaves 100x margin).
  - PE transposes of q go to a bf16 PSUM tile, 8 slices per bank, one
    [128,8,64] ACT evacuation each (instead of 32 per-slice copies).
  - software pipelining: per x piece p the PE runs [q-MMs p],
    [transposes p-1], [gram p-2] so no PE instruction waits on an
    evacuation that was just issued; sample 1's stream phase is
    interleaved with sample 0's output phase in 2-matmul blocks so the
    PE queue never head-blocks on the DVE-paced residual adds.
  - engine budget per core: PE ~88K cycles, DVE ~45us (residual adds +
    softmax), ACT ~20us (evacuations), DMA 94us (16.8MB in + 16.8MB
    out at ~358 GB/s) -- DMA is the roofline; everything else hides
    under it.
"""

import os
import sys
from contextlib import ExitStack

import ml_dtypes
import numpy as np

for _p in ("/opt/trn_rl_repo", "/root/.axon_site/_ro/trn_rl_repo"):
    if os.path.isdir(_p) and _p not in sys.path:
        sys.path.append(_p)

import concourse.bass as bass  # noqa: E402
import concourse.tile as tile  # noqa: E402
from concourse import bacc, mybir  # noqa: E402
from concourse.bass_utils import run_bass_kernel_spmd  # noqa: E402

F32 = mybir.dt.float32
F32R = mybir.dt.float32r
BF16 = mybir.dt.bfloat16
AF = mybir.ActivationFunctionType
ALU = mybir.AluOpType
AX = mybir.AxisListType

B, C, CR = 16, 512, 64
W, H = 64, 64
N = W * H  # 4096
NCORES = 8
BPC = B // NCORES  # samples per core
KC = C // 128  # 4 k-chunks of x / oc-chunks of output
NF = 512  # n-chunk (PSUM bank) width
NN = N // NF  # 8 n-chunks per sample
LF = 1024  # DMA piece width (load and store)
NL = N // LF  # 4 pieces per sample row
NPP = LF // NF  # 2 n-chunks per piece
TSL = LF // 128  # 8 transpose slices per piece
NT = N // 128  # 32 transpose slices per sample


def _build_nc():
    nc = bacc.Bacc(
        "TRN2",
        target_bir_lowering=False,
        debug=False,
        enable_asserts=True,
        num_devices=NCORES,
    )
    x_d = nc.dram_tensor("x", [BPC, C, N], F32, kind="ExternalInput").ap()
    w1t_d = nc.dram_tensor("w1t", [C, CR], F32, kind="ExternalInput").ap()
    b1c_d = nc.dram_tensor("b1c", [CR, 1], F32, kind="ExternalInput").ap()
    w2t_d = nc.dram_tensor("w2t", [CR, C], F32, kind="ExternalInput").ap()
    b2r_d = nc.dram_tensor("b2r", [1, C], F32, kind="ExternalInput").ap()
    identb_d = nc.dram_tensor("identb", [CR, CR], BF16, kind="ExternalInput").ap()
    onesb_d = nc.dram_tensor("onesb", [1, N], BF16, kind="ExternalInput").ap()
    out_d = nc.dram_tensor("out", [BPC, C, N], F32, kind="ExternalOutput").ap()

    with tile.TileContext(nc) as tc, ExitStack() as ctx:
        singles = ctx.enter_context(tc.tile_pool(name="singles", bufs=1))
        fin = ctx.enter_context(tc.tile_pool(name="fin", bufs=8))
        small = ctx.enter_context(tc.tile_pool(name="small", bufs=2))
        ps_q = ctx.enter_context(tc.tile_pool(name="ps_q", bufs=2, space="PSUM"))
        ps_tp = ctx.enter_context(tc.tile_pool(name="ps_tp", bufs=2, space="PSUM"))
        ps_att = ctx.enter_context(tc.tile_pool(name="ps_att", bufs=1, space="PSUM"))
        ps_o = ctx.enter_context(tc.tile_pool(name="ps_o", bufs=3, space="PSUM"))

        # ---------- constants / weight prep ----------
        # w1t chunks first (first q matmul needs them), then x piece 0
        # is issued before the rest of the prologue weights.
        w1t_st = singles.tile([128, KC, CR], F32, tag="w1t_st")
        nc.sync.dma_start(out=w1t_st, in_=w1t_d.rearrange("(k p) m -> p k m", k=KC))
        # x tiles: one [128, KC, 1024] 2MB dma per piece; piece (0,0) is
        # issued before the rest of the prologue so compute starts early.
        xts = [
            singles.tile([128, KC, N], BF16, tag=f"x{s}", name=f"xt{s}")
            for s in range(BPC)
        ]

        def load_piece(s, piece):
            """SWDGE load with inline f32 -> bf16 cast (gpsimd-only feature)."""
            lsl = bass.ts(piece, LF)
            nc.gpsimd.dma_start(
                out=xts[s][:, :, lsl],
                in_=x_d[s, :, lsl].rearrange("(k p) n -> p k n", p=128),
            )

        # all x pieces issue back-to-back at the head of the gpsimd queue
        for piece in range(NL):
            load_piece(0, piece)
        for piece in range(NL):
            load_piece(1, piece)

        ident = singles.tile([CR, CR], BF16, tag="ident")
        nc.sync.dma_start(out=ident, in_=identb_d)
        w1t_bf = singles.tile([128, KC, CR], BF16, tag="w1t_bf")
        nc.vector.tensor_copy(w1t_bf, w1t_st)
        b1c_sb = singles.tile([CR, 1], F32, tag="b1c")
        nc.sync.dma_start(out=b1c_sb, in_=b1c_d)
        w2t_st = singles.tile([CR, C], F32, tag="w2t_st")
        nc.sync.dma_start(out=w2t_st, in_=w2t_d)
        w2t_bf = singles.tile([CR, C], BF16, tag="w2t_bf")
        nc.vector.tensor_copy(w2t_bf, w2t_st)
        b2_st = small.tile([1, C], F32, tag="b2st")
        nc.sync.dma_start(out=b2_st, in_=b2r_d)
        # per-sample persistent tiles
        m2aug = []
        q_aug = []
        qT = []
        for s in range(BPC):
            ma = singles.tile([CR + 1, C], BF16, tag=f"m2aug{s}")
            nc.vector.tensor_copy(ma[CR : CR + 1, :], b2_st)
            m2aug.append(ma)
            qa = singles.tile([CR + 1, N], BF16, tag=f"qaug{s}")
            nc.sync.dma_start(out=qa[CR : CR + 1, :], in_=onesb_d)
            q_aug.append(qa)
            qt = singles.tile([128, NT, CR], BF16, tag=f"qT{s}", name=f"qT{s}")
            qT.append(qt)
        # warm the Exp table set early (off the critical path)
        _warm = small.tile([1, 1], F32, tag="warm")
        nc.scalar.activation(_warm, b2_st[:, 0:1], AF.Exp, bias=0.0, scale=0.0)

        state = {}

        def begin_sample(s):
            state[s] = {
                "patt": ps_att.tile([CR, CR], F32, tag="patt", name=f"patt{s}"),
            }

        def mm_chunk(s, n):
            """q[:, n] = w1 @ x into PSUM; ACT-evacuate (+b1 bias) as bf16."""
            nsl = bass.ts(n, NF)
            pq = ps_q.tile([CR, NF], F32, tag="pq", name=f"pq{s}_{n}")
            for k in range(KC):
                nc.tensor.matmul(
                    pq,
                    w1t_bf[:, k, :],
                    xts[s][:, k, nsl],
                    start=(k == 0),
                    stop=(k == KC - 1),
                )
            nc.scalar.activation(
                q_aug[s][0:CR, nsl], pq, AF.Identity, bias=b1c_sb, scale=1.0
            )

        def tp_piece(s, p):
            """PE-transpose 8 q slices of piece p into one bf16 PSUM bank."""
            tpt = ps_tp.tile([128, TSL, CR], BF16, tag="tp", name=f"tp{s}_{p}")
            for sl in range(TSL):
                t = p * TSL + sl
                nc.tensor.transpose(
                    tpt[:, sl, :],
                    q_aug[s][0:CR, t * 128 : (t + 1) * 128],
                    ident,
                )
            nc.scalar.copy(qT[s][:, p * TSL : (p + 1) * TSL, :], tpt)

        def gram_piece(s, p):
            for sl in range(TSL):
                t = p * TSL + sl
                qs = qT[s][:, t, :]
                nc.tensor.matmul(
                    state[s]["patt"], qs, qs, start=(t == 0), stop=(t == NT - 1)
                )

        def softmax_m2(s):
            """patt -> att (softmax) -> m2aug rows 0..63 = (w2 @ att)^T."""
            patt = state[s]["patt"]
            negm = small.tile([CR, 1], F32, tag="negm", name=f"negm{s}")
            nc.vector.tensor_reduce(
                out=negm, in_=patt, axis=AX.X, op=ALU.max, negate=True
            )
            shifted = small.tile([CR, CR], F32, tag="shifted", name=f"shifted{s}")
            nc.vector.tensor_scalar(
                out=shifted, in0=patt, scalar1=negm, scalar2=-80.0,
                op0=ALU.add, op1=ALU.max,
            )
            atte = small.tile([CR, CR], F32, tag="atte", name=f"atte{s}")
            ssum = small.tile([CR, 1], F32, tag="ssum", name=f"ssum{s}")
            nc.scalar.activation(
                atte, shifted, AF.Exp, bias=0.0, scale=1.0, accum_out=ssum
            )
            rsum = small.tile([CR, 1], F32, tag="rsum", name=f"rsum{s}")
            nc.vector.reciprocal(rsum, ssum)
            attn = small.tile([CR, CR], BF16, tag="attn", name=f"attn{s}")
            nc.vector.tensor_scalar_mul(attn, atte, rsum)
            # m2T = att^T @ w2^T : one matmul, att stationary
            pm2 = ps_q.tile([CR, C], F32, tag="pq", name=f"pm2{s}")
            nc.tensor.matmul(pm2, attn, w2t_bf, start=True, stop=True)
            nc.scalar.copy(m2aug[s][0:CR, :], pm2)

        def y_block(s, j):
            """y chunk: 2 matmuls + residual adds + one [128,1024] store.

            Runs at scheduler priority 0 so the store-feeding chain is
            popped the moment its dependencies resolve, instead of after
            the whole remaining stream phase."""
            oc, half = divmod(j, NL)
            osl = slice(oc * 128, (oc + 1) * 128)
            with tc.high_priority():
                f = fin.tile([128, LF], F32, tag="fin", name=f"fin{s}_{j}")
                for sub in range(NPP):
                    n = half * NPP + sub
                    nsl = bass.ts(n, NF)
                    py = ps_o.tile([128, NF], F32, tag="py", name=f"py{s}_{oc}_{n}")
                    nc.tensor.matmul(
                        py, m2aug[s][:, osl], q_aug[s][:, nsl], start=True, stop=True
                    )
                    nc.vector.tensor_add(
                        f[:, sub * NF : (sub + 1) * NF], py, xts[s][:, oc, nsl]
                    )
                nc.sync.dma_start(out=out_d[s, osl, bass.ts(half, LF)], in_=f)

        def a_blocks(s):
            """Stream-phase PE blocks for sample s, software-pipelined."""
            blocks = []
            done_tp = [False] * NL
            done_gr = [False] * NL
            for p in range(NL):
                blocks.append(lambda s=s, n=2 * p: mm_chunk(s, n))
                blocks.append(lambda s=s, n=2 * p + 1: mm_chunk(s, n))
                if p >= 1:
                    blocks.append(lambda s=s, p=p - 1: tp_piece(s, p))
                    done_tp[p - 1] = True
                if p >= 2:
                    blocks.append(lambda s=s, p=p - 2: gram_piece(s, p))
                    done_gr[p - 2] = True
            for p in range(NL):
                if not done_tp[p]:
                    blocks.append(lambda s=s, p=p: tp_piece(s, p))
            for p in range(NL):
                if not done_gr[p]:
                    blocks.append(lambda s=s, p=p: gram_piece(s, p))
            return blocks

        def warm_burst(tag, nmm=16):
            """Back-to-back dependency-free bf16 matmuls to trip the HAM
            activity monitor into K=8/8 (2.4 GHz).  Output is discarded.
            Uses the pq pool (idle between stream phases) and normal
            priority, so real work always preempts it."""
            for i in range(nmm):
                pw = ps_q.tile([CR, NF], F32, tag="pq", name=f"warm{tag}_{i}")
                nc.tensor.matmul(
                    pw, w2t_bf[:, 0:CR], w2t_bf, start=True, stop=True
                )

        # ---------- schedule ----------
        begin_sample(0)
        for blk in a_blocks(0):
            blk()
        # gapless burst right as s0's stream phase drains: HAM un-throttles
        # to 2.4 GHz while the softmax chain runs on DVE/ACT in parallel,
        # so the whole interleave phase (s1 stream + s0 output) runs warm.
        warm_burst("i", nmm=12)
        softmax_m2(0)
        begin_sample(1)
        s1_blocks = a_blocks(1)
        s0_yblocks = [lambda s=0, j=j: y_block(s, j) for j in range(16)]
        # interleave: alternate one s1 stream block with one s0 y block
        for i in range(max(len(s1_blocks), len(s0_yblocks))):
            if i < len(s1_blocks):
                s1_blocks[i]()
            if i < len(s0_yblocks):
                s0_yblocks[i]()
        softmax_m2(1)
        for j in range(16):
            y_block(1, j)

    nc.compile()
    return nc


_NC_CACHE = None


def _get_nc():
    global _NC_CACHE
    if _NC_CACHE is None:
        _NC_CACHE = _build_nc()
    return _NC_CACHE


def _as_f32(a):
    return np.ascontiguousarray(np.asarray(a, dtype=np.float32))


def run(inputs, trace=False):
    """Run on all 8 cores; returns (full output [B,C,W,H], BassKernelResults)."""
    nc = _get_nc()
    x = _as_f32(inputs["x"]).reshape(B, C, N)
    w1t = _as_f32(np.asarray(inputs["w1"]).T)  # [C, CR]
    b1c = _as_f32(inputs["b1"]).reshape(CR, 1)
    w2t = _as_f32(np.asarray(inputs["w2"]).T)  # [CR, C]
    b2r = _as_f32(inputs["b2"]).reshape(1, C)
    in_maps = [
        {
            "x": x[c * BPC : (c + 1) * BPC],
            "w1t": w1t,
            "b1c": b1c,
            "w2t": w2t,
            "b2r": b2r,
            "identb": np.eye(CR, dtype=ml_dtypes.bfloat16),
            "onesb": np.ones((1, N), dtype=ml_dtypes.bfloat16),
        }
        for c in range(NCORES)
    ]
    res = run_bass_kernel_spmd(nc, in_maps, list(range(NCORES)), trace=trace)
    out = np.concatenate([res.results[c]["out"] for c in range(NCORES)], axis=0)
    return out.reshape(B, C, W, H).astype(np.float32, copy=False), res


def kernel(**inputs):
    out, _ = run(inputs)
    return out


# revision 40
# speedup vs baseline: 1.1371x; 1.0021x over previous
"""Trainium2 Bass kernel for nn_AttentionModule (channel self-attention).

Reference computation (per batch sample b, with x: [C=512, N=4096]):
    q   = w1 @ x + b1                     # [64, 4096]
    att = softmax(q @ q.T, axis=-1)       # [64, 64]
    out = att @ q                         # [64, 4096]
    y   = w2 @ out + b2 + x               # [512, 4096]

Sharding: data-parallel over batch. B=16 samples, 8 cores, 2 samples/core.
Small weights (w1,b1,w2,b2) replicated to every core (host pre-transposes
them into matmul-friendly layouts: w1t=[C,Cr], w2t=[Cr,C], row vectors).

Key structure (vs the earlier 145us version, which was PE-bound and ran
the tensor engine at the HAM-throttled 1.2 GHz clock for most of the
kernel while the final sample's output phase trickled stores):

  - w2 @ (att @ q) == (w2 @ att) @ q: compute m2T = att^T @ w2^T with a
    single N=512 matmul (att stationary), then y = m2aug @ q_aug where
    m2aug = [m2^T; b2] (K=65).  This removes the whole out=att@q pass
    and its PSUM evacuations.
  - b1 enters q via a K=1 matmul (b1 row stationary, ones row moving),
    so the q evacuation is a pure ACT cast to bf16.
  - q, qT, att, m2, w2 all bf16 (softmax is saturated: att diagonal
    dominates by >2000, and the 2e-2 tolerance leaves 100x margin).
  - PE transposes of q go to a bf16 PSUM tile, 8 slices per bank, one
    [128,8,64] ACT evacuation each (instead of 32 per-slice copies).
  - software pipelining: per x piece p the PE runs [q-MMs p],
    [transposes p-1], [gram p-2] so no PE instruction waits on an
    evacuation that was just issued; sample 1's stream phase is
    interleaved with sample 0's output phase in 2-matmul blocks so the
    PE queue never head-blocks on the DVE-paced residual adds.
  - engine budget per core: PE ~88K cycles, DVE ~45us (residual adds +
    softmax), ACT ~20us (evacuations), DMA 94us (16.8MB in + 16.8MB
    out at ~358 GB/s) -- DMA is the roofline; everything else hides
    under it.
"""

import os
import sys
from contextlib import ExitStack

import ml_dtypes
import numpy as np

for _p in ("/opt/trn_rl_repo", "/root/.axon_site/_ro/trn_rl_repo"):
    if os.path.isdir(_p) and _p not in sys.path:
        sys.path.append(_p)

import concourse.bass as bass  # noqa: E402
import concourse.tile as tile  # noqa: E402
from concourse import bacc, mybir  # noqa: E402
from concourse.bass_utils import run_bass_kernel_spmd  # noqa: E402

F32 = mybir.dt.float32
F32R = mybir.dt.float32r
BF16 = mybir.dt.bfloat16
AF = mybir.ActivationFunctionType
ALU = mybir.AluOpType
AX = mybir.AxisListType

B, C, CR = 16, 512, 64
W, H = 64, 64
N = W * H  # 4096
NCORES = 8
BPC = B // NCORES  # samples per core
KC = C // 128  # 4 k-chunks of x / oc-chunks of output
NF = 512  # n-chunk (PSUM bank) width
NN = N // NF  # 8 n-chunks per sample
LF = 1024  # DMA piece width (load and store)
NL = N // LF  # 4 pieces per sample row
NPP = LF // NF  # 2 n-chunks per piece
TSL = LF // 128  # 8 transpose slices per piece
NT = N // 128  # 32 transpose slices per sample


def _build_nc():
    nc = bacc.Bacc(
        "TRN2",
        target_bir_lowering=False,
        debug=False,
        enable_asserts=True,
        num_devices=NCORES,
    )
    x_d = nc.dram_tensor("x", [BPC, C, N], F32, kind="ExternalInput").ap()
    w1t_d = nc.dram_tensor("w1t", [C, CR], F32, kind="ExternalInput").ap()
    b1c_d = nc.dram_tensor("b1c", [CR, 1], F32, kind="ExternalInput").ap()
    w2t_d = nc.dram_tensor("w2t", [CR, C], F32, kind="ExternalInput").ap()
    b2r_d = nc.dram_tensor("b2r", [1, C], F32, kind="ExternalInput").ap()
    identb_d = nc.dram_tensor("identb", [CR, CR], BF16, kind="ExternalInput").ap()
    onesb_d = nc.dram_tensor("onesb", [1, N], BF16, kind="ExternalInput").ap()
    out_d = nc.dram_tensor("out", [BPC, C, N], F32, kind="ExternalOutput").ap()

    with tile.TileContext(nc) as tc, ExitStack() as ctx:
        singles = ctx.enter_context(tc.tile_pool(name="singles", bufs=1))
        fin = ctx.enter_context(tc.tile_pool(name="fin", bufs=8))
        small = ctx.enter_context(tc.tile_pool(name="small", bufs=2))
        ps_q = ctx.enter_context(tc.tile_pool(name="ps_q", bufs=2, space="PSUM"))
        ps_tp = ctx.enter_context(tc.tile_pool(name="ps_tp", bufs=2, space="PSUM"))
        ps_att = ctx.enter_context(tc.tile_pool(name="ps_att", bufs=1, space="PSUM"))
        ps_o = ctx.enter_context(tc.tile_pool(name="ps_o", bufs=3, space="PSUM"))

        # ---------- constants / weight prep ----------
        # w1t chunks first (first q matmul needs them), then x piece 0
        # is issued before the rest of the prologue weights.
        w1t_st = singles.tile([128, KC, CR], F32, tag="w1t_st")
        nc.sync.dma_start(out=w1t_st, in_=w1t_d.rearrange("(k p) m -> p k m", k=KC))
        # x tiles: one [128, KC, 1024] 2MB dma per piece; piece (0,0) is
        # issued before the rest of the prologue so compute starts early.
        xts = [
            singles.tile([128, KC, N], BF16, tag=f"x{s}", name=f"xt{s}")
            for s in range(BPC)
        ]

        def load_piece(s, piece):
            """SWDGE load with inline f32 -> bf16 cast (gpsimd-only feature)."""
            lsl = bass.ts(piece, LF)
            nc.gpsimd.dma_start(
                out=xts[s][:, :, lsl],
                in_=x_d[s, :, lsl].rearrange("(k p) n -> p k n", p=128),
            )

        # all x pieces issue back-to-back at the head of the gpsimd queue
        for piece in range(NL):
            load_piece(0, piece)
        for piece in range(NL):
            load_piece(1, piece)

        ident = singles.tile([CR, CR], BF16, tag="ident")
        nc.sync.dma_start(out=ident, in_=identb_d)
        w1t_bf = singles.tile([128, KC, CR], BF16, tag="w1t_bf")
        nc.vector.tensor_copy(w1t_bf, w1t_st)
        b1c_sb = singles.tile([CR, 1], F32, tag="b1c")
        nc.sync.dma_start(out=b1c_sb, in_=b1c_d)
        w2t_st = singles.tile([CR, C], F32, tag="w2t_st")
        nc.sync.dma_start(out=w2t_st, in_=w2t_d)
        w2t_bf = singles.tile([CR, C], BF16, tag="w2t_bf")
        nc.vector.tensor_copy(w2t_bf, w2t_st)
        b2_st = small.tile([1, C], F32, tag="b2st")
        nc.sync.dma_start(out=b2_st, in_=b2r_d)
        # per-sample persistent tiles
        m2aug = []
        q_aug = []
        qT = []
        for s in range(BPC):
            ma = singles.tile([CR + 1, C], BF16, tag=f"m2aug{s}")
            nc.vector.tensor_copy(ma[CR : CR + 1, :], b2_st)
            m2aug.append(ma)
            qa = singles.tile([CR + 1, N], BF16, tag=f"qaug{s}")
            nc.sync.dma_start(out=qa[CR : CR + 1, :], in_=onesb_d)
            q_aug.append(qa)
            qt = singles.tile([128, NT, CR], BF16, tag=f"qT{s}", name=f"qT{s}")
            qT.append(qt)
        # warm the Exp table set early (off the critical path)
        _warm = small.tile([1, 1], F32, tag="warm")
        nc.scalar.activation(_warm, b2_st[:, 0:1], AF.Exp, bias=0.0, scale=0.0)

        state = {}

        def begin_sample(s):
            state[s] = {
                "patt": ps_att.tile([CR, CR], F32, tag="patt", name=f"patt{s}"),
            }

        def mm_chunk(s, n):
            """q[:, n] = w1 @ x into PSUM; ACT-evacuate (+b1 bias) as bf16."""
            nsl = bass.ts(n, NF)
            pq = ps_q.tile([CR, NF], F32, tag="pq", name=f"pq{s}_{n}")
            for k in range(KC):
                nc.tensor.matmul(
                    pq,
                    w1t_bf[:, k, :],
                    xts[s][:, k, nsl],
                    start=(k == 0),
                    stop=(k == KC - 1),
                )
            nc.scalar.activation(
                q_aug[s][0:CR, nsl], pq, AF.Identity, bias=b1c_sb, scale=1.0
            )

        def tp_piece(s, p):
            """PE-transpose 8 q slices of piece p into one bf16 PSUM bank."""
            tpt = ps_tp.tile([128, TSL, CR], BF16, tag="tp", name=f"tp{s}_{p}")
            for sl in range(TSL):
                t = p * TSL + sl
                nc.tensor.transpose(
                    tpt[:, sl, :],
                    q_aug[s][0:CR, t * 128 : (t + 1) * 128],
                    ident,
                )
            nc.scalar.copy(qT[s][:, p * TSL : (p + 1) * TSL, :], tpt)

        def gram_piece(s, p):
            for sl in range(TSL):
                t = p * TSL + sl
                qs = qT[s][:, t, :]
                nc.tensor.matmul(
                    state[s]["patt"], qs, qs, start=(t == 0), stop=(t == NT - 1)
                )

        def softmax_m2(s):
            """patt -> att (softmax) -> m2aug rows 0..63 = (w2 @ att)^T."""
            patt = state[s]["patt"]
            negm = small.tile([CR, 1], F32, tag="negm", name=f"negm{s}")
            nc.vector.tensor_reduce(
                out=negm, in_=patt, axis=AX.X, op=ALU.max, negate=True
            )
            shifted = small.tile([CR, CR], F32, tag="shifted", name=f"shifted{s}")
            nc.vector.tensor_scalar(
                out=shifted, in0=patt, scalar1=negm, scalar2=-80.0,
                op0=ALU.add, op1=ALU.max,
            )
            atte = small.tile([CR, CR], F32, tag="atte", name=f"atte{s}")
            ssum = small.tile([CR, 1], F32, tag="ssum", name=f"ssum{s}")
            nc.scalar.activation(
                atte, shifted, AF.Exp, bias=0.0, scale=1.0, accum_out=ssum
            )
            rsum = small.tile([CR, 1], F32, tag="rsum", name=f"rsum{s}")
            nc.vector.reciprocal(rsum, ssum)
            attn = small.tile([CR, CR], BF16, tag="attn", name=f"attn{s}")
            nc.vector.tensor_scalar_mul(attn, atte, rsum)
            # m2T = att^T @ w2^T : one matmul, att stationary
            pm2 = ps_q.tile([CR, C], F32, tag="pq", name=f"pm2{s}")
            nc.tensor.matmul(pm2, attn, w2t_bf, start=True, stop=True)
            nc.scalar.copy(m2aug[s][0:CR, :], pm2)

        def y_block(s, j):
            """y chunk: 2 matmuls + residual adds + one [128,1024] store.

            Runs at scheduler priority 0 so the store-feeding chain is
            popped the moment its dependencies resolve, instead of after
            the whole remaining stream phase."""
            oc, half = divmod(j, NL)
            osl = slice(oc * 128, (oc + 1) * 128)
            with tc.high_priority():
                f = fin.tile([128, LF], F32, tag="fin", name=f"fin{s}_{j}")
                for sub in range(NPP):
                    n = half * NPP + sub
                    nsl = bass.ts(n, NF)
                    py = ps_o.tile([128, NF], F32, tag="py", name=f"py{s}_{oc}_{n}")
                    nc.tensor.matmul(
                        py, m2aug[s][:, osl], q_aug[s][:, nsl], start=True, stop=True
                    )
                    nc.vector.tensor_add(
                        f[:, sub * NF : (sub + 1) * NF], py, xts[s][:, oc, nsl]
                    )
                nc.sync.dma_start(out=out_d[s, osl, bass.ts(half, LF)], in_=f)

        # Measured piece arrival times (us).  The Tile scheduler freezes the
        # per-engine instruction order from its own simulation; feeding it
        # realistic x-piece timing via tile_wait_until makes that order
        # match reality, so output-phase work lands in the real load gaps
        # instead of head-blocking behind stream work.
        ARRIVE = [10.5, 17.1, 23.7, 30.3, 37.3, 44.3, 51.3, 58.3]

        def a_blocks(s):
            """Stream-phase PE blocks for sample s, software-pipelined."""

            def w(us, fn, *a):
                def run():
                    with tc.tile_wait_until(us / 1000.0):
                        fn(*a)

                return run

            blocks = []
            done_tp = [False] * NL
            done_gr = [False] * NL
            for p in range(NL):
                g = s * NL + p
                blocks.append(w(ARRIVE[g], mm_chunk, s, 2 * p))
                blocks.append(w(ARRIVE[g], mm_chunk, s, 2 * p + 1))
                if p >= 1:
                    blocks.append(w(ARRIVE[s * NL + p - 1] + 1.2, tp_piece, s, p - 1))
                    done_tp[p - 1] = True
                if p >= 2:
                    blocks.append(w(ARRIVE[s * NL + p - 2] + 2.0, gram_piece, s, p - 2))
                    done_gr[p - 2] = True
            for p in range(NL):
                if not done_tp[p]:
                    blocks.append(w(ARRIVE[s * NL + p] + 1.2, tp_piece, s, p))
            for p in range(NL):
                if not done_gr[p]:
                    blocks.append(w(ARRIVE[s * NL + p] + 2.0, gram_piece, s, p))
            return blocks

        def warm_burst(tag, nmm=16):
            """Back-to-back dependency-free bf16 matmuls to trip the HAM
            activity monitor into K=8/8 (2.4 GHz).  Output is discarded.
            Uses the pq pool (idle between stream phases) and normal
            priority, so real work always preempts it."""
            for i in range(nmm):
                pw = ps_q.tile([CR, NF], F32, tag="pq", name=f"warm{tag}_{i}")
                nc.tensor.matmul(
                    pw, w2t_bf[:, 0:CR], w2t_bf, start=True, stop=True
                )

        # ---------- schedule ----------
        begin_sample(0)
        for blk in a_blocks(0):
            blk()
        softmax_m2(0)
        begin_sample(1)
        s1_blocks = a_blocks(1)
        s0_yblocks = [lambda s=0, j=j: y_block(s, j) for j in range(16)]
        # interleave: alternate one s1 stream block with one s0 y block
        for i in range(max(len(s1_blocks), len(s0_yblocks))):
            if i < len(s1_blocks):
                s1_blocks[i]()
            if i < len(s0_yblocks):
                s0_yblocks[i]()
        softmax_m2(1)
        for j in range(16):
            y_block(1, j)

    nc.compile()
    return nc


_NC_CACHE = None


def _get_nc():
    global _NC_CACHE
    if _NC_CACHE is None:
        _NC_CACHE = _build_nc()
    return _NC_CACHE


def _as_f32(a):
    return np.ascontiguousarray(np.asarray(a, dtype=np.float32))


def run(inputs, trace=False):
    """Run on all 8 cores; returns (full output [B,C,W,H], BassKernelResults)."""
    nc = _get_nc()
    x = _as_f32(inputs["x"]).reshape(B, C, N)
    w1t = _as_f32(np.asarray(inputs["w1"]).T)  # [C, CR]
    b1c = _as_f32(inputs["b1"]).reshape(CR, 1)
    w2t = _as_f32(np.asarray(inputs["w2"]).T)  # [CR, C]
    b2r = _as_f32(inputs["b2"]).reshape(1, C)
    in_maps = [
        {
            "x": x[c * BPC : (c + 1) * BPC],
            "w1t": w1t,
            "b1c": b1c,
            "w2t": w2t,
            "b2r": b2r,
            "identb": np.eye(CR, dtype=ml_dtypes.bfloat16),
            "onesb": np.ones((1, N), dtype=ml_dtypes.bfloat16),
        }
        for c in range(NCORES)
    ]
    res = run_bass_kernel_spmd(nc, in_maps, list(range(NCORES)), trace=trace)
    out = np.concatenate([res.results[c]["out"] for c in range(NCORES)], axis=0)
    return out.reshape(B, C, W, H).astype(np.float32, copy=False), res


def kernel(**inputs):
    out, _ = run(inputs)
    return out
